# revision 13
# baseline (speedup 1.0000x reference)
"""Trainium2 Bass kernel for nn_Decoder (RBF decoder).

Math (shapes: t (4,512,1), z (4,512,128), x (4,512,1), sigma (128,),
W (2,128), b (2,)):
    diff[b,n,m] = x[b,m] - t[b,n]                  (XD=1, sum(-1) trivial)
    K[b,n,m,c]  = exp(-0.5 * (diff/exp(sigma[c]))^2)
    y[b,m,c]    = sum_n z[b,n,c] * K[b,n,m,c]
    out[b,m,:]  = y[b,m,:] @ W.T + b

When all sigma[c] are equal (they are zeros for this problem), K is
channel-independent, so W can be folded into z up front:
    zw[b] = z[b] @ W.T            (host, (N,2) per batch — tiny)
    out[b].T = sum_n zw[b,n,:]^T K[b][n,:],  K[b] = exp(s * (x_m - t_n)^2),
    s = -0.5*exp(-2*sigma).

Device mapping (8 cores, SPMD): core k handles batch b=k//2, n-half
h=k%2 (n-slice of 256 = 2 tiles of 128 partitions). Per core:
  - x is DMA-broadcast to a (128, 512) SBUF tile.
  - ScalarE: d2 = Square(x_bcast + (-t_col))  (per-partition bias),
    then K = exp(s * d2)  (s baked as float immediate). Both functions
    live in the exp_and_others ACT table set: one table load.
  - PE: psum(2,512) += matmul(lhsT=zw_cols (128,2), rhs=K (128,512)),
    accumulated over the 2 n-tiles. Before the real matmuls, a string
    of bf16 dummy matmuls on a zeroed scratch tile runs during the
    otherwise-idle DMA-wait window to push the PE HAM clock gate to
    8/8 (2.4 GHz) so the fp32 matmuls run at full rate.
  - DVE + ScalarE each evict half of the psum -> SBUF, 2 half DMAs out
    (2,512) = out[b].T partial.
Host sums the two n-half partials per batch, transposes, adds bias b.

Sync-wait discipline: this container's walrus allows a single on_wait
per instruction ("Too many sync wait commands"), so _split_multi_waits
rewrites the scheduled BIR, hoisting extra waits onto same-engine NOPs
placed immediately before the instruction (same-engine program order
preserves semantics).

General (non-uniform) sigma falls back to grouping channels by unique
sigma value (zw_g from just that group's channels, s_g baked into a
per-group NEFF) and summing the group outputs, which is exact since the
output is linear in z. The graded instance has sigma == 0: one group.
"""

import numpy as np

B, N, M, C, Y = 4, 512, 512, 128, 2
NHALF = N // 2  # n-slice per core
NT = NHALF // 128  # n-tiles of 128 per core

_CACHE = {}


def _split_multi_waits(nc):
    import concourse.mybir as mybir

    for fn in nc.m.functions:
        for blk in fn.blocks:
            il = blk.instructions
            new = []
            for inst in il:
                si = inst.sync_info
                if si is not None and si.on_wait is not None and len(si.on_wait) > 1:
                    waits = list(si.on_wait)
                    for j, w in enumerate(waits[:-1]):
                        new.append(
                            mybir.InstNoOp(
                                name=f"{inst.name}-w{j}",
                                engine=inst.engine,
                                sync_info=mybir.SyncInfo(on_wait=[w], on_update=[]),
                                bass_nofuse=True,
                            )
                        )
                    si.on_wait = [waits[-1]]
                    inst.sync_info = si
                new.append(inst)
            il[:] = new


def build_bass(s: float):
    """Build the per-core Bass module; `s` (= -0.5*exp(-2*sigma)) is baked
    into the exp activation as a float immediate."""
    import concourse.bass as bass
    import concourse.mybir as mybir
    import concourse.tile as tile

    f32 = mybir.dt.float32
    bf16 = mybir.dt.bfloat16
    nc = bass.Bass()
    xv = nc.dram_tensor("xv", (M,), f32, kind="ExternalInput")
    # tz = [-t col per nt | zw cols per nt]: (128, NT + NT*Y)
    tz = nc.dram_tensor("tz", (128, NT * (1 + Y)), f32, kind="ExternalInput")
    o = nc.dram_tensor("o", (Y, M), f32, kind="ExternalOutput")

    N_WARM = 8  # dummy bf16 matmuls to warm the PE HAM gate (~3.5us busy)

    with tile.TileContext(nc) as tc:
        with (
            tc.tile_pool(name="const", bufs=1) as cpool,
            tc.tile_pool(name="work", bufs=2) as work,
            tc.tile_pool(name="psum", bufs=1, space="PSUM") as psum,
            tc.tile_pool(name="wpsum", bufs=1, space="PSUM") as wpsum,
        ):
            # PE warm-up: zeroed bf16 scratch, matmuls with no input deps.
            scr = cpool.tile([128, M], bf16)
            nc.vector.memset(scr, 0.0)
            w_ps = wpsum.tile([128, M], f32)
            for _ in range(N_WARM):
                nc.tensor.matmul(
                    w_ps, lhsT=scr[:, 0:128], rhs=scr, start=True, stop=True
                )

            xb_sb = cpool.tile([128, M], f32)
            xsrc = bass.AP(tensor=xv, offset=0, ap=[[0, 128], [1, M]])
            nc.sync.dma_start(out=xb_sb, in_=xsrc)
            tz_sb = cpool.tile([128, NT * (1 + Y)], f32)
            nc.sync.dma_start(out=tz_sb, in_=tz[:])

            o_ps = psum.tile([Y, M], f32)
            for nt in range(NT):
                d2_sb = work.tile([128, M], f32, tag="d2")
                nc.scalar.activation(
                    d2_sb,
                    xb_sb,
                    mybir.ActivationFunctionType.Square,
                    bias=tz_sb[:, nt : nt + 1],
                )
                k_sb = work.tile([128, M], f32, tag="k")
                nc.scalar.activation(
                    k_sb, d2_sb, mybir.ActivationFunctionType.Exp, scale=float(s)
                )
                nc.tensor.matmul(
                    o_ps,
                    lhsT=tz_sb[:, NT + nt * Y : NT + (nt + 1) * Y],
                    rhs=k_sb,
                    start=(nt == 0),
                    stop=(nt == NT - 1),
                )
            o_sb = cpool.tile([Y, M], f32)
            half = M // 2
            nc.vector.tensor_copy(o_sb[:, 0:half], o_ps[:, 0:half])
            nc.scalar.copy(o_sb[:, half:M], o_ps[:, half:M])
            nc.sync.dma_start(out=o[:, 0:half], in_=o_sb[:, 0:half])
            nc.sync.dma_start(out=o[:, half:M], in_=o_sb[:, half:M])
    _split_multi_waits(nc)
    return nc


def _get_nc(s: float):
    key = ("nc", float(s))
    if key not in _CACHE:
        _CACHE[key] = build_bass(s)
    return _CACHE[key]


def _in_maps_for_group(t, x, zw):
    """Build the 8 per-core input dicts for one sigma-group.

    zw: (B, N, Y) = z[:, :, group] @ W[:, group].T
    """
    in_maps = []
    for core in range(8):
        b, h = core // 2, core % 2
        tb = t[b, h * NHALF : (h + 1) * NHALF, 0]
        tzm = np.empty((128, NT * (1 + Y)), np.float32)
        for nt in range(NT):
            lo = h * NHALF + nt * 128
            tzm[:, nt] = -tb[nt * 128 : (nt + 1) * 128]
            tzm[:, NT + nt * Y : NT + (nt + 1) * Y] = zw[b, lo : lo + 128, :]
        in_maps.append(
            {
                "xv": np.ascontiguousarray(x[b, :, 0]),
                "tz": tzm,
            }
        )
    return in_maps


def _run_group(t, x, zw, s, trace=False):
    from concourse.bass_utils import run_bass_kernel_spmd

    res = run_bass_kernel_spmd(
        _get_nc(s),
        _in_maps_for_group(t, x, zw),
        core_ids=list(range(8)),
        trace=trace,
    )
    out = np.zeros((B, M, Y), np.float32)
    for b in range(B):
        acc = res.results[2 * b]["o"] + res.results[2 * b + 1]["o"]  # (Y, M)
        out[b] = acc.T
    return out, res


def kernel(**inputs):
    t = np.asarray(inputs["t"], np.float32)
    z = np.asarray(inputs["z"], np.float32)
    x = np.asarray(inputs["x"], np.float32)
    sigma = np.asarray(inputs["sigma"], np.float32)
    W = np.asarray(inputs["W"], np.float32)
    bias = np.asarray(inputs["b"], np.float32)

    trace = bool(_CACHE.pop("trace", False))
    out = np.zeros((B, M, Y), np.float32)
    if np.all(sigma == sigma[0]):
        s = -0.5 * float(np.exp(-2.0 * sigma[0]))
        zw = z @ W.T  # (B, N, Y)
        grp_out, res = _run_group(t, x, zw.astype(np.float32), s, trace=trace)
        out += grp_out
        _CACHE["last_results"] = res
    else:
        for val in np.unique(sigma):
            idx = np.nonzero(sigma == val)[0]
            zw = z[:, :, idx] @ W[:, idx].T
            s = -0.5 * float(np.exp(-2.0 * val))
            grp_out, res = _run_group(t, x, zw.astype(np.float32), s, trace=False)
            out += grp_out
    out += bias[None, None, :]
    return out
